# revision 1
# baseline (speedup 1.0000x reference)
"""AdaptiveMultiWIRE on 8 TRN2 NeuronCores.

Sharding: C=16 channels over 8 cores (2 channels/core), zero collectives.
All index gathers (indices/model_idx/bias_idx) happen host-side in numpy.

Device layout (per core, per channel):
  - activations feature-major: X tiles {re[0:128], re[128:181], im[0:128],
    im[128:181]} = {XA[128], XB1[53], XC[128], XB2[53]}, N on the free dim.
  - fp16 matmuls (full TensorE rate), fp32 psum accumulate.
  - each layer's matmul writes one psum "wave" per feature chunk:
    [P, 4*512] fp32 = 4 groups (g0..g3) as free-dim slices -> every
    activation op is partition-aligned across groups.
  - group pre-scaling + all per-feature biases folded into the fp16 weights
    (biases enter via one rank-4 "group mask" matmul per group per wave):
      g0 = (OMEGA/2pi) * (la.re + b)        phase in "turns"
      g1 = SCALE * (la.im + b) + OMEGA/2S   complete-the-square form
      g2 = SCALE * (lb.re + b)
      g3 = SCALE * (lb.im + b)
  - trig via exact fp32 magic-number range reduction (k = round(g0),
    f = g0 - k), then Sin(2pi f); cos via half-angle 1 - 2 sin(pi f)^2 with
    the re-output stored negated (next layer's re-input weight rows are
    negated host-side to compensate).
  - "wide act": per-wave psum-bound ops (squares, range reduction) write
    slices of 4-ntile-wide SBUF tiles; the transcendentals, sums, and
    combines then run at FD=2048, amortizing the ~800ns/instr ScalarE
    overhead and batching activation-table usage (Sin/Exp live in
    different table sets).
"""

import numpy as np

C, N, H, OUT, NIN, NSRC, NB = 16, 8192, 181, 3, 2, 32, 8
OMEGA, SCALE = 30.0, 10.0
NCORES, CPC = 8, 2
PI = float(np.pi)
KHI, KLO = 128, H - 128          # 128 / 53 feature chunks
TW = 512                         # psum wave width (one PSUM bank)
NB_NT = 4                        # ntiles batched per wide-act phase
WB = NB_NT * TW                  # 2048
NBATCH = N // WB                 # 4
R2 = OMEGA / (2.0 * PI)
S0 = SCALE / R2
EBIAS = OMEGA * OMEGA / (4.0 * SCALE * SCALE)   # 2.25
MAGIC = 12582912.0               # 1.5 * 2^23 forces round-to-int in fp32 adds

_GRAPH = None


def _build_graph():
    import concourse.mybir as mybir
    from concourse import bacc
    from concourse.tile import TileContext

    dt = mybir.dt
    f16, f32 = dt.float16, dt.float32
    Alu = mybir.AluOpType
    Act = mybir.ActivationFunctionType

    # Bacc: its compile() runs generate_event_semaphores(), required for the
    # TRN2 one-sync-wait-per-instruction ISA constraint.
    nc = bacc.Bacc()
    xa_d = nc.declare_dram_parameter("xa", [CPC, NIN + 1, N], f16, isOutput=False)
    w0_d = nc.declare_dram_parameter("w0", [CPC, NIN + 1, 2 * H], f16, isOutput=False)
    w1_d = nc.declare_dram_parameter("w1", [CPC, 2 * H + 1, 4 * H], f16, isOutput=False)
    w2_d = nc.declare_dram_parameter("w2", [CPC, 2 * H + 1, 4 * H], f16, isOutput=False)
    wf_d = nc.declare_dram_parameter("wf", [CPC, 2 * H + 1, OUT], f16, isOutput=False)
    out_d = nc.declare_dram_parameter("out", [CPC, OUT, N], f16, isOutput=True)

    KROWS = [(0, KHI), (KHI, H), (H, H + KHI), (H + KHI, 2 * H + 1)]

    with TileContext(nc) as tc:
        with (
            tc.tile_pool(name="wpool", bufs=1) as wpool,
            tc.tile_pool(name="xpool", bufs=1) as xpool,
            tc.tile_pool(name="spool", bufs=1) as spool,
            tc.tile_pool(name="lpool", bufs=1) as lpool,
            tc.tile_pool(name="psum", bufs=2, space="PSUM") as pp,
        ):
            # ---- persistent loads -------------------------------------
            wts, wfts = [], []
            xat, w0t = [], []
            for ch in range(CPC):
                t = wpool.tile([NIN + 1, N], f16, tag=f"xa{ch}", name=f"xa{ch}")
                nc.sync.dma_start(out=t[:], in_=xa_d[ch])
                xat.append(t)
                t = wpool.tile([NIN + 1, 2 * H], f16, tag=f"w0{ch}",
                               name=f"w0{ch}")
                nc.sync.dma_start(out=t[:], in_=w0_d[ch])
                w0t.append(t)
                per_layer = []
                for li, wd in ((1, w1_d), (2, w2_d)):
                    tiles = []
                    for ki, (r0, r1) in enumerate(KROWS):
                        t = wpool.tile([r1 - r0, 4 * H], f16, tag=f"w{li}{ch}k{ki}")
                        nc.sync.dma_start(out=t[:], in_=wd[ch, r0:r1, :])
                        tiles.append(t)
                    per_layer.append(tiles)
                wts.append(per_layer)
                tiles = []
                for ki, (r0, r1) in enumerate(KROWS):
                    t = wpool.tile([r1 - r0, OUT], f16, tag=f"wf{ch}k{ki}")
                    nc.sync.dma_start(out=t[:], in_=wf_d[ch, r0:r1, :])
                    tiles.append(t)
                wfts.append(tiles)

            def layer_mm(ps, rhs_tiles, mlo, mhi, n_groups):
                """All matmuls for one psum wave (one feature chunk);
                biases ride the ones-rows baked into the rhs tiles."""
                nk = len(rhs_tiles)
                for g in range(n_groups):
                    sl = slice(g * TW, (g + 1) * TW)
                    for ki, (wt, xt, xsl) in enumerate(rhs_tiles):
                        nc.tensor.matmul(ps[:, sl],
                                         lhsT=wt[:, g * H + mlo:g * H + mhi],
                                         rhs=xt[:, xsl],
                                         start=(ki == 0), stop=(ki == nk - 1))

            def early_act(ps, P, n_groups, fw, u0w, uGw, ni):
                """Per-wave psum-bound ops -> slices of 4-ntile-wide tiles."""
                k1 = spool.tile([P, TW], f32, tag="k1")
                nc.vector.tensor_scalar(k1[:], ps[:, 0:TW], MAGIC,
                                        MAGIC, Alu.add, Alu.subtract)
                nc.vector.scalar_tensor_tensor(fw[:, ni * TW:(ni + 1) * TW],
                                               k1[:], -1.0, ps[:, 0:TW],
                                               Alu.mult, Alu.add)
                if n_groups == 4:
                    nc.scalar.activation(u0w[:, ni * TW:(ni + 1) * TW],
                                         ps[:, 0:TW], Act.Square,
                                         bias=0.0, scale=S0)
                    # one 3D-AP Square covers g1/g2/g3 (contiguous in psum;
                    # uGw layout [P, (g-1) groups, NB_NT*TW])
                    out3 = uGw[:].rearrange("p (g w) -> p g w", w=WB)
                    ps3 = ps[:, TW:4 * TW].rearrange("p (g w) -> p g w", w=TW)
                    nc.scalar.activation(out3[:, :, ni * TW:(ni + 1) * TW],
                                         ps3, Act.Square, bias=0.0, scale=1.0)
                else:
                    nc.scalar.activation(u0w[:, ni * TW:(ni + 1) * TW],
                                         ps[:, 0:TW], Act.Square,
                                         bias=0.0, scale=S0)
                    nc.scalar.activation(uGw[:, ni * TW:(ni + 1) * TW],
                                         ps[:, TW:2 * TW], Act.Square,
                                         bias=0.0, scale=1.0)

            # late act is phase-split so both feature chunks' Sin instrs run
            # back-to-back before any Exp (Sin and Exp live in different
            # activation-table sets; ~1.3us per table switch)
            def late_trig(P, fw, hl):
                s = lpool.tile([P, WB], f16, tag=f"s{hl}", name=f"s{hl}")
                sh = lpool.tile([P, WB], f16, tag=f"sh{hl}", name=f"sh{hl}")
                nc.scalar.activation(s[:], fw[:], Act.Sin, bias=0.0,
                                     scale=2 * PI)
                nc.scalar.activation(sh[:], fw[:], Act.Sin, bias=0.0, scale=PI)
                return s, sh

            def late_exp(P, n_groups, u0w, uGw, hl):
                w = lpool.tile([P, WB], f16, tag=f"w{hl}", name=f"w{hl}")
                if n_groups == 4:
                    v1 = lpool.tile([P, WB], f16, tag=f"v1{hl}", name=f"v1{hl}")
                    v2 = lpool.tile([P, WB], f16, tag=f"v2{hl}", name=f"v2{hl}")
                    nc.gpsimd.tensor_tensor(v1[:], uGw[:, 0:WB],
                                            uGw[:, WB:2 * WB], Alu.add)
                    nc.gpsimd.tensor_tensor(v2[:], uGw[:, 2 * WB:3 * WB],
                                            u0w[:], Alu.add)
                    # E = exp(-w) then carries a spurious e^-EBIAS factor;
                    # the host scales the next layer's weights by e^EBIAS.
                    nc.vector.tensor_tensor(w[:], v1[:], v2[:], Alu.add)
                else:
                    nc.vector.tensor_tensor(w[:], u0w[:], uGw[:], Alu.add)
                E = lpool.tile([P, WB], f16, tag=f"E{hl}", name=f"E{hl}")
                nc.scalar.activation(E[:], w[:], Act.Exp, bias=0.0, scale=-1.0)
                return E

            def late_combine(P, E, s, sh, xre_w, xim_w, hl):
                q = lpool.tile([P, WB], f16, tag=f"v1{hl}", name=f"q{hl}")
                nc.vector.scalar_tensor_tensor(q[:], sh[:], 2.0, sh[:],
                                               Alu.mult, Alu.mult)
                # xre' = (2 sh^2 - 1) E = -E cos ; next layer re-rows negated
                nc.vector.scalar_tensor_tensor(xre_w[:], q[:], -1.0, E[:],
                                               Alu.add, Alu.mult)
                nc.vector.tensor_tensor(xim_w[:], E[:], s[:], Alu.mult)

            def late_act2(fwh, u0h, uGh, fwl, u0l, uGl, ng,
                          XAo, XCo, XB1o, XB2o):
                sh_, shh = late_trig(KHI, fwh, "h")
                sl_, shl = late_trig(KLO, fwl, "l")
                Eh = late_exp(KHI, ng, u0h, uGh, "h")
                El = late_exp(KLO, ng, u0l, uGl, "l")
                late_combine(KHI, Eh, sh_, shh, XAo, XCo, "h")
                late_combine(KLO, El, sl_, shl, XB1o, XB2o, "l")

            def alloc_x(ch):
                xb2 = xpool.tile([KLO + 1, WB], f16, tag=f"XB2{ch}",
                                 name=f"XB2{ch}")
                # full-tile memset (partition-offset memsets are illegal); the
                # combine overwrites rows 0..KLO-1, leaving the ones row
                nc.vector.memset(xb2[:], 1.0)
                return (xpool.tile([KHI, WB], f16, tag=f"XA{ch}", name=f"XA{ch}"),
                        xpool.tile([KLO, WB], f16, tag=f"XB1{ch}", name=f"XB1{ch}"),
                        xpool.tile([KHI, WB], f16, tag=f"XC{ch}", name=f"XC{ch}"),
                        xb2)

            def alloc_wide(tagp, P, n_groups):
                fw = spool.tile([P, WB], f16, tag=f"fw{tagp}", name=f"fw{tagp}")
                u0w = spool.tile([P, WB], f16, tag=f"u0w{tagp}", name=f"u0w{tagp}")
                uGw = spool.tile([P, (n_groups - 1) * WB], f16,
                                 tag=f"uGw{tagp}", name=f"uGw{tagp}")
                return fw, u0w, uGw

            # ---- main loop: 4 batches of 4 ntiles ---------------------
            # Channels are interleaved per layer: PE executes in issue order,
            # so ch1's independent layer-li matmuls sit right behind ch0's
            # and keep the PE busy while ch0's activation phase runs.
            for nb in range(NBATCH):
                bsl = slice(nb * WB, (nb + 1) * WB)
                X = {}
                for li in (0, 1, 2):
                    for ch in range(CPC):
                        if li == 0:
                            rhs = [(w0t[ch], xat[ch], None)]
                            ng = 2
                        else:
                            XA, XB1, XC, XB2 = X[ch]
                            wk = wts[ch][li - 1]
                            rhs = [(wk[0], XA, None), (wk[1], XB1, None),
                                   (wk[2], XC, None), (wk[3], XB2, None)]
                            ng = 4
                        X[ch] = alloc_x(ch)
                        XA, XB1, XC, XB2 = X[ch]
                        fwh, u0h, uGh = alloc_wide("h", KHI, ng)
                        fwl, u0l, uGl = alloc_wide("l", KLO, ng)
                        for ni in range(NB_NT):
                            if li == 0:
                                wsl = slice((nb * NB_NT + ni) * TW,
                                            (nb * NB_NT + ni + 1) * TW)
                            else:
                                wsl = slice(ni * TW, (ni + 1) * TW)
                            rhs_n = [(wt, xt, wsl) for wt, xt, _ in rhs]
                            ps = pp.tile([KHI, ng * TW], f32, tag="wav")
                            layer_mm(ps, rhs_n, 0, KHI, ng)
                            early_act(ps, KHI, ng, fwh, u0h, uGh, ni)
                            psl = pp.tile([KLO, ng * TW], f32, tag="wav")
                            layer_mm(psl, rhs_n, KHI, H, ng)
                            early_act(psl, KLO, ng, fwl, u0l, uGl, ni)
                        late_act2(fwh, u0h, uGh, fwl, u0l, uGl, ng,
                                  XA, XC, XB1, XB2[0:KLO, :])
                # ---------- final linear (M=3, real part) ----------
                for ch in range(CPC):
                    XA, XB1, XC, XB2 = X[ch]
                    ob = lpool.tile([OUT, WB], f16, tag=f"ob{ch}")
                    for ni in range(NB_NT):
                        wsl = slice(ni * TW, (ni + 1) * TW)
                        psf = pp.tile([OUT, TW], f32, tag="wav")
                        for ki, xt in enumerate((XA, XB1, XC, XB2)):
                            nc.tensor.matmul(psf[:], lhsT=wfts[ch][ki][:],
                                             rhs=xt[:, wsl],
                                             start=(ki == 0), stop=(ki == 3))
                        nc.scalar.activation(ob[:, wsl], psf[:], Act.Identity,
                                             bias=0.0, scale=1.0)
                    nc.sync.dma_start(out=out_d[ch, :, bsl], in_=ob[:])
    nc.finalize()
    return nc


def _get_graph():
    global _GRAPH
    if _GRAPH is None:
        _GRAPH = _build_graph()
    return _GRAPH


def _pack_inputs(inp, indices, model_idx, bias_idx, W0a, b0a, W0b, b0b,
                 W1a, b1a, W1b, b1b, W2a, b2a, W2b, b2b, Wf, bf):
    """Host-side gather + weight packing. Returns in_maps for 8 cores."""
    cplx = lambda a: a[..., 0] + 1j * a[..., 1]

    def pack_hidden(Wa, Wb, insc, ba, bb):
        g0r, g0i = R2 * Wa.real, -R2 * Wa.imag
        g1r, g1i = SCALE * Wa.imag, SCALE * Wa.real
        g2r, g2i = SCALE * Wb.real, -SCALE * Wb.imag
        g3r, g3i = SCALE * Wb.imag, SCALE * Wb.real
        # re-input rows negated: the device stores xre' = -xre (sign trick
        # from the 1-2sin^2 cosine path); insc compensates the e^-EBIAS
        # factor the previous hidden layer's activations carry
        Wre = -insc * np.concatenate([g0r, g1r, g2r, g3r], 1)
        Wim = insc * np.concatenate([g0i, g1i, g2i, g3i], 1)
        # per-group bias row, streamed against the ones-row in the rhs
        brow = np.concatenate([
            R2 * ba.real, SCALE * ba.imag + OMEGA / (2 * SCALE),
            SCALE * bb.real, SCALE * bb.imag])[None, :]
        return np.concatenate(
            [Wre[:KHI], Wre[KHI:], Wim[:KHI], Wim[KHI:], brow],
            0).astype(np.float16)

    in_maps = []
    for core in range(NCORES):
        m = {k: [] for k in ("xa", "w0", "w1", "w2", "wf")}
        for j in range(CPC):
            c = core * CPC + j
            mi, bi = int(model_idx[c]), int(bias_idx[c])
            x = inp[int(indices[c])]          # [N, NIN]
            m["xa"].append(np.concatenate(
                [x.T, np.ones((1, N), np.float32)], 0).astype(np.float16))
            w0row = np.concatenate(
                [R2 * b0a[bi], SCALE * b0b[bi]])[None, :]
            w0blk = np.concatenate([np.concatenate(
                [R2 * W0a[mi], SCALE * W0b[mi]], 1), w0row], 0)
            m["w0"].append(w0blk.astype(np.float16))
            Wa1, Wb1 = cplx(W1a[mi]), cplx(W1b[mi])
            Wa2, Wb2 = cplx(W2a[mi]), cplx(W2b[mi])
            ba1, bb1 = cplx(b1a[bi]), cplx(b1b[bi])
            ba2, bb2 = cplx(b2a[bi]), cplx(b2b[bi])
            m["w1"].append(pack_hidden(Wa1, Wb1, 1.0, ba1, bb1))
            m["w2"].append(pack_hidden(Wa2, Wb2, float(np.exp(EBIAS)),
                                       ba2, bb2))
            Wfc = cplx(Wf[mi])
            wfblk = float(np.exp(EBIAS)) * np.concatenate(
                [-Wfc.real, -Wfc.imag], 0)
            wfblk = np.concatenate(
                [wfblk[:KHI], wfblk[KHI:H], wfblk[H:H + KHI], wfblk[H + KHI:],
                 cplx(bf[bi]).real[None, :]], 0).astype(np.float16)
            m["wf"].append(wfblk)
        packed = {k: np.stack(v) for k, v in m.items()}
        in_maps.append(packed)
    return in_maps


def kernel(**inputs):
    inp = np.asarray(inputs["inp"], np.float32)
    args = {k: np.asarray(v) for k, v in inputs.items()}
    in_maps = _pack_inputs(
        inp, args["indices"], args["model_idx"], args["bias_idx"],
        *[np.asarray(args[k], np.float32) for k in
          ("W0a", "b0a", "W0b", "b0b", "W1a", "b1a", "W1b", "b1b",
           "W2a", "b2a", "W2b", "b2b", "Wf", "bf")])
    from concourse.bass_utils import run_bass_kernel_spmd
    nc = _get_graph()
    res = run_bass_kernel_spmd(nc, in_maps, core_ids=list(range(NCORES)))
    out = np.empty((1, C, N, OUT), np.float32)
    for core in range(NCORES):
        o = np.asarray(res.results[core]["out"])   # [CPC, OUT, N] fp16
        for j in range(CPC):
            out[0, core * CPC + j] = o[j].T.astype(np.float32)
    return out


if __name__ == "__main__":
    import reference
    ins = {k: np.asarray(v) for k, v in reference.setup_inputs().items()}
    got = kernel(**ins)
    exp = np.asarray(reference.reference(**ins))
    rel = np.linalg.norm(got - exp) / np.linalg.norm(exp)
    print("Relative error:", rel)



# revision 4
# speedup vs baseline: 1.1402x; 1.1402x over previous
"""AdaptiveMultiWIRE on 8 TRN2 NeuronCores — point-major rewrite.

Sharding: C=16 channels over 8 cores (2 channels/core), zero collectives.
All index gathers happen host-side in numpy.

Core idea vs the feature-major v1: activations are the matmul's
STATIONARY operand (lhsT, feature-major K-chunks 128/128/107) and the
packed weights STREAM as rhs [K, 724], so each psum wave is
[128 points, 724 group-columns].  Consequences:

  - PE: 3 K-chunks instead of 4 (363 rows pack into 3 partition blocks
    with zero M-waste: points are 128-aligned).  ~0.94 K-efficiency vs
    ~0.50 for the old {128,53}x{128,53} tiling.
  - every elementwise op runs on [128, free] slices with ZERO partition
    waste (the old layout paid 2x: a 53-partition instr costs the same
    as a 128-partition one).  Group alignment is free-dim slicing.
  - the activation outputs are point-major, so each layer ends with 48
    PE transposes (128x<=128 through an identity rhs, fp16 psum) plus 3
    psum->SBUF evictions to rebuild the feature-major lhsT for the next
    layer.

Math per layer (same fused Gabor chain as v1):
  g0 = (OMEGA/2pi)*(la.re + b)            phase in "turns"
  g1 = S*(la.im + b) + OMEGA/2S           complete-the-square form
  g2 = S*(lb.re + b);  g3 = S*(lb.im + b)
  f  = g0 - round(g0)  (fp32 magic-number round on DVE)
  E  = exp(-((S0*g0)^2 + g1^2 + g2^2 + g3^2))   (carries e^-EBIAS)
  s = Sin(2pi f),  sh = Sin(pi f)  (HW Sin is only valid on [-pi, pi])
  xre' = (sh^2 - 1/2) * E  = -cos(2pi g0)*E/2   (half-angle cosine; the
         -2x is folded into the next layer's re-input weight rows)
  xim' = E * s
"""

import numpy as np

C, N, H, OUT, NIN, NSRC, NB = 16, 8192, 181, 3, 2, 32, 8
OMEGA, SCALE = 30.0, 10.0
NCORES, CPC = 8, 2
PI = float(np.pi)
R2 = OMEGA / (2.0 * PI)          # turns per unit la.re
S0 = SCALE / R2
EBIAS = OMEGA * OMEGA / (4.0 * SCALE * SCALE)   # 2.25
MAGIC = 12582912.0               # 1.5*2^23: fp32 add/sub rounds to int
PB = 2048                        # points per batch
NBATCH = N // PB                 # 4
PC = 128                         # points per chunk (psum partition dim)
NPC = PB // PC                   # 16 chunks per batch
NW = 4 * H                       # 724 hidden wave columns
NW0 = 2 * H                      # 362 layer-0 wave columns
KC = [(0, 128), (128, 256), (256, 363)]   # K row chunks of 2H+1

_GRAPH = None


def _build_graph():
    import concourse.mybir as mybir
    from concourse import bacc
    from concourse.tile import TileContext

    dt = mybir.dt
    f16, f32 = dt.float16, dt.float32
    Alu = mybir.AluOpType
    Act = mybir.ActivationFunctionType

    nc = bacc.Bacc()
    xa_d = nc.declare_dram_parameter("xa", [CPC, 3, N], f16, isOutput=False)
    w0_d = nc.declare_dram_parameter("w0", [CPC, 3, NW0], f16, isOutput=False)
    w1_d = nc.declare_dram_parameter("w1", [CPC, 2 * H + 1, NW], f16,
                                     isOutput=False)
    w2_d = nc.declare_dram_parameter("w2", [CPC, 2 * H + 1, NW], f16,
                                     isOutput=False)
    wf_d = nc.declare_dram_parameter("wf", [CPC, 2 * H + 1, OUT], f16,
                                     isOutput=False)
    idn_d = nc.declare_dram_parameter("idn", [128, 128], f16, isOutput=False)
    out_d = nc.declare_dram_parameter("out", [CPC, OUT, N], f16, isOutput=True)

    with TileContext(nc) as tc:
        with (
            tc.tile_pool(name="wpool", bufs=1) as wpool,
            tc.tile_pool(name="xpool", bufs=1) as xpool,
            tc.tile_pool(name="apool", bufs=2) as apool,
            tc.tile_pool(name="spool", bufs=1) as spool,
            tc.tile_pool(name="kpool", bufs=2) as kpool,
            tc.tile_pool(name="psum", bufs=2, space="PSUM") as pp,
        ):
            # ---- persistent loads ------------------------------------
            idn = wpool.tile([128, 128], f16, tag="idn", name="idn")
            nc.sync.dma_start(out=idn[:], in_=idn_d[:])
            w0t, wts, wfts = [], [], []
            for ch in range(CPC):
                t = wpool.tile([3, NW0], f16, tag=f"w0{ch}", name=f"w0{ch}")
                nc.sync.dma_start(out=t[:], in_=w0_d[ch])
                w0t.append(t)
                per_layer = []
                for li, wd in ((1, w1_d), (2, w2_d)):
                    tiles = []
                    for ki, (r0, r1) in enumerate(KC):
                        t = wpool.tile([r1 - r0, NW], f16, tag=f"w{li}{ch}k{ki}")
                        nc.sync.dma_start(out=t[:], in_=wd[ch, r0:r1, :])
                        tiles.append(t)
                    per_layer.append(tiles)
                wts.append(per_layer)
                tiles = []
                for ki, (r0, r1) in enumerate(KC):
                    t = wpool.tile([r1 - r0, OUT], f16, tag=f"wf{ch}k{ki}")
                    nc.sync.dma_start(out=t[:], in_=wf_d[ch, r0:r1, :])
                    tiles.append(t)
                wfts.append(tiles)

            def mm_early(ch, li, xan, X):
                """Matmuls + psum-bound early ops for one channel-layer.
                Returns (fw, u0w, sqw) wide tiles."""
                gw = H if li == 0 else 3 * H       # square-section width
                fw = spool.tile([128, NPC * H], f16, tag=f"fw{ch}")
                u0w = spool.tile([128, NPC * H], f16, tag=f"u0w{ch}")
                sqw = spool.tile([128, NPC * 3 * H], f16, tag=f"sqw{ch}")
                for t in range(NPC // 2):
                    ps = pp.tile([128, 2048], f32, tag="wav")
                    for slot in (0, 1):
                        pc = 2 * t + slot
                        o = slot * 1024
                        if li == 0:
                            nc.tensor.matmul(
                                ps[:, o:o + NW0],
                                lhsT=xan[:, pc * PC:(pc + 1) * PC],
                                rhs=w0t[ch][:], start=True, stop=True)
                        else:
                            wk = wts[ch][li - 1]
                            T0, T1, T2 = X
                            for ki, xt in enumerate((T0, T1, T2)):
                                lhs = xt[:, pc * PC:(pc + 1) * PC]
                                nc.tensor.matmul(ps[:, o:o + 512],
                                                 lhsT=lhs, rhs=wk[ki][:, 0:512],
                                                 start=(ki == 0), stop=(ki == 2))
                                nc.tensor.matmul(ps[:, o + 512:o + NW],
                                                 lhsT=lhs, rhs=wk[ki][:, 512:NW],
                                                 start=(ki == 0), stop=(ki == 2))
                    ps3 = ps[:].rearrange("p (s w) -> p s w", w=1024)
                    g0 = ps3[:, :, 0:H]
                    k1 = kpool.tile([128, 2 * H], f32, tag=f"k1{ch}")
                    nc.vector.tensor_scalar(k1[:], g0, MAGIC, MAGIC,
                                            Alu.add, Alu.subtract)
                    nc.vector.scalar_tensor_tensor(
                        fw[:, t * 2 * H:(t + 1) * 2 * H], k1[:], -1.0, g0,
                        Alu.mult, Alu.add)
                    nc.scalar.activation(u0w[:, t * 2 * H:(t + 1) * 2 * H],
                                         g0, Act.Square, bias=0.0, scale=S0)
                    nc.scalar.activation(
                        sqw[:, t * 2 * gw:(t + 1) * 2 * gw],
                        ps3[:, :, H:H + gw], Act.Square, bias=0.0, scale=1.0)
                return fw, u0w, sqw

            def adds_trig(ch, li, fw, u0w, sqw):
                """v-adds (DVE/Pool) + the two Sins (ScalarE)."""
                W = NPC * H
                s = spool.tile([128, W], f16, tag=f"s{ch}")
                sh = spool.tile([128, W], f16, tag=f"sh{ch}")
                if li == 0:
                    wv = spool.tile([128, W], f16, tag=f"v1{ch}")
                    nc.gpsimd.tensor_tensor(wv[:], u0w[:], sqw[:, 0:W], Alu.add)
                else:
                    sq3 = sqw[:].rearrange("p (n g) -> p n g", g=3 * H)
                    v1 = spool.tile([128, W], f16, tag=f"v1{ch}")
                    v2 = spool.tile([128, W], f16, tag=f"v2{ch}")
                    nc.vector.tensor_tensor(v1[:], sq3[:, :, 0:H],
                                            sq3[:, :, H:2 * H], Alu.add)
                    nc.gpsimd.tensor_tensor(v2[:], sq3[:, :, 2 * H:3 * H],
                                            u0w[:], Alu.add)
                    wv = spool.tile([128, W], f16, tag=f"sqw{ch}")
                    nc.gpsimd.tensor_tensor(wv[:], v1[:], v2[:], Alu.add)
                nc.scalar.activation(s[:], fw[:], Act.Sin, bias=0.0,
                                     scale=2 * PI)
                nc.scalar.activation(sh[:], fw[:], Act.Sin, bias=0.0, scale=PI)
                return s, sh, wv

            def expph(ch, wv):
                E = spool.tile([128, NPC * H], f16, tag=f"E{ch}")
                nc.scalar.activation(E[:], wv[:], Act.Exp, bias=0.0, scale=-1.0)
                return E

            def combine_transpose(ch, s, sh, E):
                """c2/xre/xim into xw, then 48 transposes + 3 evictions
                into the next layer's feature-major T tiles."""
                c2 = spool.tile([128, NPC * H], f16, tag=f"v1{ch}")
                nc.gpsimd.tensor_tensor(c2[:], sh[:], sh[:], Alu.mult)
                # xw aliases the sqw buffer (dead after the v-adds/wv)
                xw = spool.tile([128, NPC * 363], f16, tag=f"sqw{ch}")
                x3 = xw[:].rearrange("p (n g) -> p n g", g=363)
                nc.vector.scalar_tensor_tensor(x3[:, :, 0:H], c2[:], 0.5,
                                               E[:], Alu.subtract, Alu.mult)
                nc.gpsimd.tensor_tensor(x3[:, :, H:2 * H], E[:], s[:], Alu.mult)
                nc.vector.memset(x3[:, :, 2 * H:363], 1.0)
                T0 = xpool.tile([128, PB], f16, tag=f"T0{ch}", name=f"T0{ch}")
                T1 = xpool.tile([128, PB], f16, tag=f"T1{ch}", name=f"T1{ch}")
                T2 = xpool.tile([107, PB], f16, tag=f"T2{ch}", name=f"T2{ch}")
                tps = []
                for ki, (c0, c1) in enumerate(KC):
                    tp = pp.tile([c1 - c0, PB], f16, tag="wav")
                    for pc in range(NPC):
                        nc.tensor.transpose(
                            tp[:, pc * PC:(pc + 1) * PC],
                            xw[:, pc * 363 + c0:pc * 363 + c1], idn[:])
                    tps.append(tp)
                    # evict the first psum before the 3rd transpose set
                    # needs its rotated buffer back
                    if ki == 1:
                        nc.vector.tensor_copy(T0[:], tps[0][:])
                        nc.scalar.activation(T1[:], tps[1][:], Act.Identity,
                                             bias=0.0, scale=1.0)
                nc.vector.tensor_copy(T2[:], tps[2][:])
                return T0, T1, T2

            # ---- main loop -------------------------------------------
            X = {ch: None for ch in range(CPC)}
            for nb in range(NBATCH):
                xan = {}
                for ch in range(CPC):
                    t = apool.tile([3, PB], f16, tag=f"xa{ch}")
                    nc.sync.dma_start(out=t[:],
                                      in_=xa_d[ch, :, nb * PB:(nb + 1) * PB])
                    xan[ch] = t
                for li in (0, 1, 2):
                    early = {}
                    for ch in range(CPC):
                        early[ch] = mm_early(ch, li, xan[ch], X[ch])
                    mid = {}
                    for ch in range(CPC):
                        mid[ch] = adds_trig(ch, li, *early[ch])
                    Ee = {}
                    for ch in range(CPC):
                        Ee[ch] = expph(ch, mid[ch][2])
                    for ch in range(CPC):
                        s, sh, _ = mid[ch]
                        X[ch] = combine_transpose(ch, s, sh, Ee[ch])
                # ---------- final linear (M=3, real part) ----------
                for ch in range(CPC):
                    T0, T1, T2 = X[ch]
                    psf = pp.tile([OUT, PB], f32, tag="wav")
                    for ni in range(PB // 512):
                        sl = slice(ni * 512, (ni + 1) * 512)
                        for ki, xt in enumerate((T0, T1, T2)):
                            nc.tensor.matmul(psf[:, sl], lhsT=wfts[ch][ki][:],
                                             rhs=xt[:, sl],
                                             start=(ki == 0), stop=(ki == 2))
                    ob = spool.tile([OUT, PB], f16, tag=f"ob{ch}")
                    if ch == 0:
                        nc.vector.tensor_copy(ob[:], psf[:])
                    else:
                        nc.scalar.activation(ob[:], psf[:], Act.Identity,
                                             bias=0.0, scale=1.0)
                    nc.sync.dma_start(out=out_d[ch, :, nb * PB:(nb + 1) * PB],
                                      in_=ob[:])
    nc.finalize()
    return nc


def _get_graph():
    global _GRAPH
    if _GRAPH is None:
        _GRAPH = _build_graph()
    return _GRAPH


def _pack_inputs(inp, indices, model_idx, bias_idx, W0a, b0a, W0b, b0b,
                 W1a, b1a, W1b, b1b, W2a, b2a, W2b, b2b, Wf, bf):
    """Host-side gather + weight packing. Returns in_maps for 8 cores.

    Weight rows are input features in T-tile order [xre'(181); xim'(181);
    ones]; columns are the 4 group outputs [g0|g1|g2|g3].  alpha/beta are
    the affine factors mapping stored activations to true ones:
    x.re = alpha*xre', x.im = beta*xim'.
    """
    cplx = lambda a: a[..., 0] + 1j * a[..., 1]
    idn = np.eye(128, dtype=np.float16)

    def pack_hidden(Wa, Wb, ba, bb, alpha, beta):
        re_rows = np.concatenate([
            R2 * alpha * Wa.real, SCALE * alpha * Wa.imag,
            SCALE * alpha * Wb.real, SCALE * alpha * Wb.imag], axis=1)
        im_rows = np.concatenate([
            -R2 * beta * Wa.imag, SCALE * beta * Wa.real,
            -SCALE * beta * Wb.imag, SCALE * beta * Wb.real], axis=1)
        ones_row = np.concatenate([
            R2 * ba.real, SCALE * ba.imag + OMEGA / (2 * SCALE),
            SCALE * bb.real, SCALE * bb.imag])[None, :]
        return np.concatenate([re_rows, im_rows, ones_row],
                              axis=0).astype(np.float16)

    a0, b0c = -2.0, 1.0                          # L0 -> L1 factors
    a1, b1c = -2.0 * np.exp(EBIAS), np.exp(EBIAS)  # L1 -> L2, L2 -> final

    in_maps = []
    for core in range(NCORES):
        m = {k: [] for k in ("xa", "w0", "w1", "w2", "wf")}
        for j in range(CPC):
            c = core * CPC + j
            mi, bi = int(model_idx[c]), int(bias_idx[c])
            x = inp[int(indices[c])]          # [N, NIN]
            m["xa"].append(np.concatenate(
                [x.T, np.ones((1, N), np.float32)], 0).astype(np.float16))
            w0blk = np.concatenate([
                np.concatenate([R2 * W0a[mi], SCALE * W0b[mi]], axis=1),
                np.concatenate([R2 * b0a[bi], SCALE * b0b[bi]])[None, :],
            ], axis=0)
            m["w0"].append(w0blk.astype(np.float16))
            m["w1"].append(pack_hidden(cplx(W1a[mi]), cplx(W1b[mi]),
                                       cplx(b1a[bi]), cplx(b1b[bi]), a0, b0c))
            m["w2"].append(pack_hidden(cplx(W2a[mi]), cplx(W2b[mi]),
                                       cplx(b2a[bi]), cplx(b2b[bi]), a1, b1c))
            Wfc, bfc = cplx(Wf[mi]), cplx(bf[bi])
            wfblk = np.concatenate([
                a1 * Wfc.real, -b1c * Wfc.imag, bfc.real[None, :]],
                axis=0).astype(np.float16)
            m["wf"].append(wfblk)
        packed = {k: np.stack(v) for k, v in m.items()}
        packed["idn"] = idn
        in_maps.append(packed)
    return in_maps


def kernel(**inputs):
    inp = np.asarray(inputs["inp"], np.float32)
    args = {k: np.asarray(v) for k, v in inputs.items()}
    in_maps = _pack_inputs(
        inp, args["indices"], args["model_idx"], args["bias_idx"],
        *[np.asarray(args[k], np.float32) for k in
          ("W0a", "b0a", "W0b", "b0b", "W1a", "b1a", "W1b", "b1b",
           "W2a", "b2a", "W2b", "b2b", "Wf", "bf")])
    from concourse.bass_utils import run_bass_kernel_spmd
    nc = _get_graph()
    res = run_bass_kernel_spmd(nc, in_maps, core_ids=list(range(NCORES)))
    out = np.empty((1, C, N, OUT), np.float32)
    for core in range(NCORES):
        o = np.asarray(res.results[core]["out"])   # [CPC, OUT, N] fp16
        for j in range(CPC):
            out[0, core * CPC + j] = o[j].T.astype(np.float32)
    return out


if __name__ == "__main__":
    import jax
    import reference
    cpu = jax.devices("cpu")[0]
    with jax.default_device(cpu):
        ins = {k: np.asarray(v) for k, v in reference.setup_inputs().items()}
        exp = np.asarray(reference.reference(
            **{k: jax.device_put(v, cpu) for k, v in ins.items()}))
    got = kernel(**ins)
    rel = np.linalg.norm(got - exp) / np.linalg.norm(exp)
    print("Relative error:", rel)


# revision 13
# speedup vs baseline: 1.2398x; 1.0873x over previous
"""AdaptiveMultiWIRE on 8 TRN2 NeuronCores — point-major rewrite.

Sharding: C=16 channels over 8 cores (2 channels/core), zero collectives.
All index gathers happen host-side in numpy.

Core idea vs the feature-major v1: activations are the matmul's
STATIONARY operand (lhsT, feature-major K-chunks 128/128/107) and the
packed weights STREAM as rhs [K, 724], so each psum wave is
[128 points, 724 group-columns].  Consequences:

  - PE: 3 K-chunks instead of 4 (363 rows pack into 3 partition blocks
    with zero M-waste: points are 128-aligned).  ~0.94 K-efficiency vs
    ~0.50 for the old {128,53}x{128,53} tiling.
  - every elementwise op runs on [128, free] slices with ZERO partition
    waste (the old layout paid 2x: a 53-partition instr costs the same
    as a 128-partition one).  Group alignment is free-dim slicing.
  - the activation outputs are point-major, so each layer ends with 48
    PE transposes (128x<=128 through an identity rhs, fp16 psum) plus 3
    psum->SBUF evictions to rebuild the feature-major lhsT for the next
    layer.

Math per layer (same fused Gabor chain as v1):
  g0 = (OMEGA/2pi)*(la.re + b)            phase in "turns"
  g1 = S*(la.im + b) + OMEGA/2S           complete-the-square form
  g2 = S*(lb.re + b);  g3 = S*(lb.im + b)
  f  = g0 - round(g0)  (fp32 magic-number round on DVE)
  E  = exp(-((S0*g0)^2 + g1^2 + g2^2 + g3^2))   (carries e^-EBIAS)
  s = Sin(2pi f),  sh = Sin(pi f)  (HW Sin is only valid on [-pi, pi])
  xre' = (sh^2 - 1/2) * E  = -cos(2pi g0)*E/2   (half-angle cosine; the
         -2x is folded into the next layer's re-input weight rows)
  xim' = E * s
"""

import numpy as np

C, N, H, OUT, NIN, NSRC, NB = 16, 8192, 181, 3, 2, 32, 8
OMEGA, SCALE = 30.0, 10.0
NCORES, CPC = 8, 2
PI = float(np.pi)
R2 = OMEGA / (2.0 * PI)          # turns per unit la.re
S0 = SCALE / R2
EBIAS = OMEGA * OMEGA / (4.0 * SCALE * SCALE)   # 2.25
MAGIC = 12582912.0               # 1.5*2^23: fp32 add/sub rounds to int
PB = 2048                        # points per batch
NBATCH = N // PB                 # 4
PC = 128                         # points per chunk (psum partition dim)
NPC = PB // PC                   # 16 chunks per batch
NW = 5 * H                       # 905 hidden wave columns (g0..g3 + g4=S0*g0)
NW0 = 3 * H                      # 543 layer-0 wave columns (g0, g2', g4')
KC = [(0, 128), (128, 256), (256, 363)]   # K row chunks of 2H+1

_GRAPH = None


def _build_graph():
    import concourse.mybir as mybir
    from concourse import bacc
    from concourse.tile import TileContext

    dt = mybir.dt
    f16, f32 = dt.float16, dt.float32
    Alu = mybir.AluOpType
    Act = mybir.ActivationFunctionType

    nc = bacc.Bacc()
    xa_d = nc.declare_dram_parameter("xa", [CPC, 3, N], f16, isOutput=False)
    w0_d = nc.declare_dram_parameter("w0", [CPC, 3, NW0], f16, isOutput=False)
    w1_d = nc.declare_dram_parameter("w1", [CPC, 2 * H + 1, NW], f16,
                                     isOutput=False)
    w2_d = nc.declare_dram_parameter("w2", [CPC, 2 * H + 1, NW], f16,
                                     isOutput=False)
    wf_d = nc.declare_dram_parameter("wf", [CPC, 2 * H + 1, OUT], f16,
                                     isOutput=False)
    idn_d = nc.declare_dram_parameter("idn", [128, 128], f16, isOutput=False)
    out_d = nc.declare_dram_parameter("out", [CPC, OUT, N], f16, isOutput=True)

    with TileContext(nc) as tc:
        with (
            tc.tile_pool(name="wpool", bufs=1) as wpool,
            tc.tile_pool(name="xpool", bufs=1) as xpool,
            tc.tile_pool(name="apool", bufs=2) as apool,
            tc.tile_pool(name="spool", bufs=1) as spool,
            tc.tile_pool(name="kpool", bufs=2) as kpool,
            tc.tile_pool(name="psum", bufs=2, space="PSUM") as pp,
        ):
            # ---- persistent loads ------------------------------------
            idn = wpool.tile([128, 128], f16, tag="idn", name="idn")
            nc.sync.dma_start(out=idn[:], in_=idn_d[:])
            w0t, wts, wfts = [], [], []
            for ch in range(CPC):
                t = wpool.tile([3, NW0], f16, tag=f"w0{ch}", name=f"w0{ch}")
                nc.sync.dma_start(out=t[:], in_=w0_d[ch])
                w0t.append(t)
                per_layer = []
                for li, wd in ((1, w1_d), (2, w2_d)):
                    tiles = []
                    for ki, (r0, r1) in enumerate(KC):
                        t = wpool.tile([r1 - r0, NW], f16, tag=f"w{li}{ch}k{ki}")
                        nc.sync.dma_start(out=t[:], in_=wd[ch, r0:r1, :])
                        tiles.append(t)
                    per_layer.append(tiles)
                wts.append(per_layer)
                tiles = []
                for ki, (r0, r1) in enumerate(KC):
                    t = wpool.tile([r1 - r0, OUT], f16, tag=f"wf{ch}k{ki}")
                    nc.sync.dma_start(out=t[:], in_=wf_d[ch, r0:r1, :])
                    tiles.append(t)
                wfts.append(tiles)

            def mm_early(ch, li, xan, X):
                """Matmuls + psum-bound early ops for one channel-layer.
                Returns (fw, sqw) wide tiles."""
                gw = 2 * H if li == 0 else 4 * H   # square-section width
                fw = spool.tile([128, NPC * H], f16, tag=f"fw{ch}")
                sqw = spool.tile([128, NPC * 4 * H], f16, tag=f"sqw{ch}")
                for t in range(NPC // 2):
                    ps = pp.tile([128, 2048], f32, tag="wav")
                    for slot in (0, 1):
                        pc = 2 * t + slot
                        o = slot * 1024
                        if li == 0:
                            lhs = xan[:, pc * PC:(pc + 1) * PC]
                            nc.tensor.matmul(ps[:, o:o + 512], lhsT=lhs,
                                             rhs=w0t[ch][:, 0:512],
                                             start=True, stop=True)
                            nc.tensor.matmul(ps[:, o + 512:o + NW0], lhsT=lhs,
                                             rhs=w0t[ch][:, 512:NW0],
                                             start=True, stop=True)
                        else:
                            wk = wts[ch][li - 1]
                            T0, T1, T2 = X
                            for ki, xt in enumerate((T0, T1, T2)):
                                lhs = xt[:, pc * PC:(pc + 1) * PC]
                                nc.tensor.matmul(ps[:, o:o + 512],
                                                 lhsT=lhs, rhs=wk[ki][:, 0:512],
                                                 start=(ki == 0), stop=(ki == 2))
                                nc.tensor.matmul(ps[:, o + 512:o + NW],
                                                 lhsT=lhs, rhs=wk[ki][:, 512:NW],
                                                 start=(ki == 0), stop=(ki == 2))
                    ps3 = ps[:].rearrange("p (s w) -> p s w", w=1024)
                    g0 = ps3[:, :, 0:H]
                    k1 = kpool.tile([128, 2 * H], f32, tag=f"k1{ch}")
                    nc.vector.tensor_scalar(k1[:], g0, MAGIC, MAGIC,
                                            Alu.add, Alu.subtract)
                    nc.vector.scalar_tensor_tensor(
                        fw[:, t * 2 * H:(t + 1) * 2 * H], k1[:], -1.0, g0,
                        Alu.mult, Alu.add)
                    nc.scalar.activation(
                        sqw[:, t * 2 * gw:(t + 1) * 2 * gw],
                        ps3[:, :, H:H + gw], Act.Square, bias=0.0, scale=1.0)
                return fw, sqw

            def adds_trig(ch, li, fw, sqw):
                """v-adds (DVE/Pool) + the two Sins (ScalarE)."""
                W = NPC * H
                s = spool.tile([128, W], f16, tag=f"s{ch}")
                sh = spool.tile([128, W], f16, tag=f"sh{ch}")
                if li == 0:
                    sq3 = sqw[:, 0:NPC * 2 * H].rearrange("p (n g) -> p n g",
                                                          g=2 * H)
                    wv = spool.tile([128, W], f16, tag=f"v1{ch}")
                    nc.gpsimd.tensor_tensor(wv[:], sq3[:, :, 0:H],
                                            sq3[:, :, H:2 * H], Alu.add)
                else:
                    sq3 = sqw[:].rearrange("p (n g) -> p n g", g=4 * H)
                    v1 = spool.tile([128, W], f16, tag=f"v1{ch}")
                    v2 = spool.tile([128, W], f16, tag=f"v2{ch}")
                    nc.vector.tensor_tensor(v1[:], sq3[:, :, 0:H],
                                            sq3[:, :, H:2 * H], Alu.add)
                    nc.gpsimd.tensor_tensor(v2[:], sq3[:, :, 2 * H:3 * H],
                                            sq3[:, :, 3 * H:4 * H], Alu.add)
                    wv = spool.tile([128, W], f16, tag=f"sqw{ch}")
                    nc.gpsimd.tensor_tensor(wv[:], v1[:], v2[:], Alu.add)
                nc.scalar.activation(s[:], fw[:], Act.Sin, bias=0.0,
                                     scale=2 * PI)
                nc.scalar.activation(sh[:], fw[:], Act.Sin, bias=0.0, scale=PI)
                return s, sh, wv

            def expph(ch, wv):
                E = spool.tile([128, NPC * H], f16, tag=f"E{ch}")
                nc.scalar.activation(E[:], wv[:], Act.Exp, bias=0.0, scale=-1.0)
                return E

            def combine_transpose(ch, s, sh, E):
                """c2/xre/xim into xw, then 48 transposes + 3 evictions
                into the next layer's feature-major T tiles."""
                c2 = spool.tile([128, NPC * H], f16, tag=f"v1{ch}")
                nc.gpsimd.tensor_tensor(c2[:], sh[:], sh[:], Alu.mult)
                # xw aliases the sqw buffer (dead after the v-adds/wv)
                xw = spool.tile([128, NPC * 363], f16, tag=f"sqw{ch}")
                x3 = xw[:].rearrange("p (n g) -> p n g", g=363)
                nc.vector.scalar_tensor_tensor(x3[:, :, 0:H], c2[:], 0.5,
                                               E[:], Alu.subtract, Alu.mult)
                nc.gpsimd.tensor_tensor(x3[:, :, H:2 * H], E[:], s[:], Alu.mult)
                nc.vector.memset(x3[:, :, 2 * H:363], 1.0)
                T0 = xpool.tile([128, PB], f16, tag=f"T0{ch}", name=f"T0{ch}")
                T1 = xpool.tile([128, PB], f16, tag=f"T1{ch}", name=f"T1{ch}")
                T2 = xpool.tile([107, PB], f16, tag=f"T2{ch}", name=f"T2{ch}")
                tps = []
                for ki, (c0, c1) in enumerate(KC):
                    tp = pp.tile([c1 - c0, PB], f16, tag="wav")
                    for pc in range(NPC):
                        nc.tensor.transpose(
                            tp[:, pc * PC:(pc + 1) * PC],
                            xw[:, pc * 363 + c0:pc * 363 + c1], idn[:])
                    tps.append(tp)
                    # evict the first psum before the 3rd transpose set
                    # needs its rotated buffer back
                    if ki == 1:
                        nc.vector.tensor_copy(T0[:], tps[0][:])
                        nc.vector.tensor_copy(T1[:], tps[1][:])
                nc.vector.tensor_copy(T2[:], tps[2][:])
                return T0, T1, T2

            # ---- main loop -------------------------------------------
            def load_xa(nb):
                d = {}
                for ch in range(CPC):
                    t = apool.tile([3, PB], f16, tag=f"xa{ch}")
                    nc.sync.dma_start(out=t[:],
                                      in_=xa_d[ch, :, nb * PB:(nb + 1) * PB])
                    d[ch] = t
                return d

            X = {ch: None for ch in range(CPC)}
            xan = load_xa(0)
            for nb in range(NBATCH):
                for li in (0, 1, 2):
                    early = {}
                    for ch in range(CPC):
                        early[ch] = mm_early(ch, li, xan[ch], X[ch])
                    if li == 0 and nb + 1 < NBATCH:
                        xan_next = load_xa(nb + 1)
                    mid = {}
                    for ch in range(CPC):
                        mid[ch] = adds_trig(ch, li, *early[ch])
                    Ee = {}
                    for ch in range(CPC):
                        Ee[ch] = expph(ch, mid[ch][2])
                    for ch in range(CPC):
                        s, sh, _ = mid[ch]
                        X[ch] = combine_transpose(ch, s, sh, Ee[ch])
                # ---------- final linear (M=3, real part) ----------
                for ch in range(CPC):
                    T0, T1, T2 = X[ch]
                    psf = pp.tile([OUT, PB], f32, tag="wav")
                    for ni in range(PB // 512):
                        sl = slice(ni * 512, (ni + 1) * 512)
                        for ki, xt in enumerate((T0, T1, T2)):
                            nc.tensor.matmul(psf[:, sl], lhsT=wfts[ch][ki][:],
                                             rhs=xt[:, sl],
                                             start=(ki == 0), stop=(ki == 2))
                    ob = spool.tile([OUT, PB], f16, tag=f"ob{ch}")
                    nc.vector.tensor_copy(ob[:], psf[:])
                    nc.sync.dma_start(out=out_d[ch, :, nb * PB:(nb + 1) * PB],
                                      in_=ob[:])
                if nb + 1 < NBATCH:
                    xan = xan_next
    nc.finalize()
    return nc


def _get_graph():
    global _GRAPH
    if _GRAPH is None:
        _GRAPH = _build_graph()
    return _GRAPH


def _pack_inputs(inp, indices, model_idx, bias_idx, W0a, b0a, W0b, b0b,
                 W1a, b1a, W1b, b1b, W2a, b2a, W2b, b2b, Wf, bf):
    """Host-side gather + weight packing. Returns in_maps for 8 cores.

    Weight rows are input features in T-tile order [xre'(181); xim'(181);
    ones]; columns are the 4 group outputs [g0|g1|g2|g3].  alpha/beta are
    the affine factors mapping stored activations to true ones:
    x.re = alpha*xre', x.im = beta*xim'.
    """
    cplx = lambda a: a[..., 0] + 1j * a[..., 1]
    idn = np.eye(128, dtype=np.float16)

    def pack_hidden(Wa, Wb, ba, bb, alpha, beta):
        # columns [g0 | g1 | g2 | g3 | g4] with g4 = S0*g0 (so the
        # sum-of-squares pass covers (S*la.re)^2 too, no separate u0)
        re_rows = np.concatenate([
            R2 * alpha * Wa.real, SCALE * alpha * Wa.imag,
            SCALE * alpha * Wb.real, SCALE * alpha * Wb.imag,
            SCALE * alpha * Wa.real], axis=1)
        im_rows = np.concatenate([
            -R2 * beta * Wa.imag, SCALE * beta * Wa.real,
            -SCALE * beta * Wb.imag, SCALE * beta * Wb.real,
            -SCALE * beta * Wa.imag], axis=1)
        ones_row = np.concatenate([
            R2 * ba.real, SCALE * ba.imag + OMEGA / (2 * SCALE),
            SCALE * bb.real, SCALE * bb.imag, SCALE * ba.real])[None, :]
        return np.concatenate([re_rows, im_rows, ones_row],
                              axis=0).astype(np.float16)

    a0, b0c = -2.0, 1.0                          # L0 -> L1 factors
    a1, b1c = -2.0 * np.exp(EBIAS), np.exp(EBIAS)  # L1 -> L2, L2 -> final

    in_maps = []
    for core in range(NCORES):
        m = {k: [] for k in ("xa", "w0", "w1", "w2", "wf")}
        for j in range(CPC):
            c = core * CPC + j
            mi, bi = int(model_idx[c]), int(bias_idx[c])
            x = inp[int(indices[c])]          # [N, NIN]
            m["xa"].append(np.concatenate(
                [x.T, np.ones((1, N), np.float32)], 0).astype(np.float16))
            w0blk = np.concatenate([
                np.concatenate([R2 * W0a[mi], SCALE * W0b[mi],
                                SCALE * W0a[mi]], axis=1),
                np.concatenate([R2 * b0a[bi], SCALE * b0b[bi],
                                SCALE * b0a[bi]])[None, :],
            ], axis=0)
            m["w0"].append(w0blk.astype(np.float16))
            m["w1"].append(pack_hidden(cplx(W1a[mi]), cplx(W1b[mi]),
                                       cplx(b1a[bi]), cplx(b1b[bi]), a0, b0c))
            m["w2"].append(pack_hidden(cplx(W2a[mi]), cplx(W2b[mi]),
                                       cplx(b2a[bi]), cplx(b2b[bi]), a1, b1c))
            Wfc, bfc = cplx(Wf[mi]), cplx(bf[bi])
            wfblk = np.concatenate([
                a1 * Wfc.real, -b1c * Wfc.imag, bfc.real[None, :]],
                axis=0).astype(np.float16)
            m["wf"].append(wfblk)
        packed = {k: np.stack(v) for k, v in m.items()}
        packed["idn"] = idn
        in_maps.append(packed)
    return in_maps


def kernel(**inputs):
    inp = np.asarray(inputs["inp"], np.float32)
    args = {k: np.asarray(v) for k, v in inputs.items()}
    in_maps = _pack_inputs(
        inp, args["indices"], args["model_idx"], args["bias_idx"],
        *[np.asarray(args[k], np.float32) for k in
          ("W0a", "b0a", "W0b", "b0b", "W1a", "b1a", "W1b", "b1b",
           "W2a", "b2a", "W2b", "b2b", "Wf", "bf")])
    from concourse.bass_utils import run_bass_kernel_spmd
    nc = _get_graph()
    res = run_bass_kernel_spmd(nc, in_maps, core_ids=list(range(NCORES)))
    out = np.empty((1, C, N, OUT), np.float32)
    for core in range(NCORES):
        o = np.asarray(res.results[core]["out"])   # [CPC, OUT, N] fp16
        for j in range(CPC):
            out[0, core * CPC + j] = o[j].T.astype(np.float32)
    return out


if __name__ == "__main__":
    import jax
    import reference
    cpu = jax.devices("cpu")[0]
    with jax.default_device(cpu):
        ins = {k: np.asarray(v) for k, v in reference.setup_inputs().items()}
        exp = np.asarray(reference.reference(
            **{k: jax.device_put(v, cpu) for k, v in ins.items()}))
    got = kernel(**ins)
    rel = np.linalg.norm(got - exp) / np.linalg.norm(exp)
    print("Relative error:", rel)


# revision 16
# speedup vs baseline: 1.2626x; 1.0184x over previous
"""AdaptiveMultiWIRE on 8 TRN2 NeuronCores — point-major rewrite.

Sharding: C=16 channels over 8 cores (2 channels/core), zero collectives.
All index gathers happen host-side in numpy.

Core idea vs the feature-major v1: activations are the matmul's
STATIONARY operand (lhsT, feature-major K-chunks 128/128/107) and the
packed weights STREAM as rhs [K, 724], so each psum wave is
[128 points, 724 group-columns].  Consequences:

  - PE: 3 K-chunks instead of 4 (363 rows pack into 3 partition blocks
    with zero M-waste: points are 128-aligned).  ~0.94 K-efficiency vs
    ~0.50 for the old {128,53}x{128,53} tiling.
  - every elementwise op runs on [128, free] slices with ZERO partition
    waste (the old layout paid 2x: a 53-partition instr costs the same
    as a 128-partition one).  Group alignment is free-dim slicing.
  - the activation outputs are point-major, so each layer ends with 48
    PE transposes (128x<=128 through an identity rhs, fp16 psum) plus 3
    psum->SBUF evictions to rebuild the feature-major lhsT for the next
    layer.

Math per layer (same fused Gabor chain as v1):
  g0 = (OMEGA/2pi)*(la.re + b)            phase in "turns"
  g1 = S*(la.im + b) + OMEGA/2S           complete-the-square form
  g2 = S*(lb.re + b);  g3 = S*(lb.im + b)
  f  = g0 - round(g0)  (fp32 magic-number round on DVE)
  E  = exp(-((S0*g0)^2 + g1^2 + g2^2 + g3^2))   (carries e^-EBIAS)
  s = Sin(2pi f),  sh = Sin(pi f)  (HW Sin is only valid on [-pi, pi])
  xre' = (sh^2 - 1/2) * E  = -cos(2pi g0)*E/2   (half-angle cosine; the
         -2x is folded into the next layer's re-input weight rows)
  xim' = E * s
"""

import numpy as np

C, N, H, OUT, NIN, NSRC, NB = 16, 8192, 181, 3, 2, 32, 8
OMEGA, SCALE = 30.0, 10.0
NCORES, CPC = 8, 2
PI = float(np.pi)
R2 = OMEGA / (2.0 * PI)          # turns per unit la.re
S0 = SCALE / R2
EBIAS = OMEGA * OMEGA / (4.0 * SCALE * SCALE)   # 2.25
MAGIC = 12582912.0               # 1.5*2^23: fp32 add/sub rounds to int
PB = 2048                        # points per batch
NBATCH = N // PB                 # 4
PC = 128                         # points per chunk (psum partition dim)
NPC = PB // PC                   # 16 chunks per batch
NW = 5 * H                       # 905 hidden wave columns (g0..g3 + g4=S0*g0)
NW0 = 3 * H                      # 543 layer-0 wave columns (g0, g2', g4')
KC = [(0, 128), (128, 256), (256, 363)]   # K row chunks of 2H+1

_GRAPH = None


def _build_graph():
    import concourse.mybir as mybir
    from concourse import bacc
    from concourse.tile import TileContext

    dt = mybir.dt
    f16, f32 = dt.float16, dt.float32
    Alu = mybir.AluOpType
    Act = mybir.ActivationFunctionType

    nc = bacc.Bacc()
    xa_d = nc.declare_dram_parameter("xa", [CPC, 3, N], f16, isOutput=False)
    w0_d = nc.declare_dram_parameter("w0", [CPC, 3, NW0], f16, isOutput=False)
    w1_d = nc.declare_dram_parameter("w1", [CPC, 2 * H + 1, NW], f16,
                                     isOutput=False)
    w2_d = nc.declare_dram_parameter("w2", [CPC, 2 * H + 1, NW], f16,
                                     isOutput=False)
    wf_d = nc.declare_dram_parameter("wf", [CPC, 2 * H + 1, OUT], f16,
                                     isOutput=False)
    idn_d = nc.declare_dram_parameter("idn", [128, 128], f16, isOutput=False)
    out_d = nc.declare_dram_parameter("out", [CPC, OUT, N], f16, isOutput=True)

    with TileContext(nc) as tc:
        with (
            tc.tile_pool(name="wpool", bufs=1) as wpool,
            tc.tile_pool(name="xpool", bufs=1) as xpool,
            tc.tile_pool(name="apool", bufs=2) as apool,
            tc.tile_pool(name="spool", bufs=1) as spool,
            tc.tile_pool(name="kpool", bufs=2) as kpool,
            tc.tile_pool(name="psum", bufs=2, space="PSUM") as pp,
        ):
            # ---- persistent loads ------------------------------------
            idn = wpool.tile([128, 128], f16, tag="idn", name="idn")
            nc.sync.dma_start(out=idn[:], in_=idn_d[:])
            w0t, wts, wfts = [], [], []
            for ch in range(CPC):
                t = wpool.tile([3, NW0], f16, tag=f"w0{ch}", name=f"w0{ch}")
                nc.sync.dma_start(out=t[:], in_=w0_d[ch])
                w0t.append(t)
                per_layer = []
                for li, wd in ((1, w1_d), (2, w2_d)):
                    tiles = []
                    for ki, (r0, r1) in enumerate(KC):
                        t = wpool.tile([r1 - r0, NW], f16, tag=f"w{li}{ch}k{ki}")
                        nc.sync.dma_start(out=t[:], in_=wd[ch, r0:r1, :])
                        tiles.append(t)
                    per_layer.append(tiles)
                wts.append(per_layer)
                tiles = []
                for ki, (r0, r1) in enumerate(KC):
                    t = wpool.tile([r1 - r0, OUT], f16, tag=f"wf{ch}k{ki}")
                    nc.sync.dma_start(out=t[:], in_=wf_d[ch, r0:r1, :])
                    tiles.append(t)
                wfts.append(tiles)

            def mm_early(ch, li, xan, X):
                """Matmuls + psum-bound early ops + per-tile v-adds +
                inline half-batch Sins + wide wv + half-batch Exps.
                Returns (s, sh, E) wide tiles."""
                gw = 2 * H if li == 0 else 4 * H   # square-section width
                W = NPC * H
                HW2 = W // 2
                fw = spool.tile([128, W], f16, tag=f"fw{ch}")
                sqw = spool.tile([128, NPC * 4 * H], f16, tag=f"sqw{ch}")
                s = spool.tile([128, W], f16, tag=f"s{ch}")
                sh = spool.tile([128, W], f16, tag=f"sh{ch}")
                if li > 0:
                    v1w = spool.tile([128, W], f16, tag=f"v1{ch}")
                    v2w = spool.tile([128, W], f16, tag=f"v2{ch}")
                for t in range(NPC // 2):
                    ps = pp.tile([128, 2048], f32, tag="wav")
                    for slot in (0, 1):
                        pc = 2 * t + slot
                        o = slot * 1024
                        if li == 0:
                            lhs = xan[:, pc * PC:(pc + 1) * PC]
                            nc.tensor.matmul(ps[:, o:o + 512], lhsT=lhs,
                                             rhs=w0t[ch][:, 0:512],
                                             start=True, stop=True)
                            nc.tensor.matmul(ps[:, o + 512:o + NW0], lhsT=lhs,
                                             rhs=w0t[ch][:, 512:NW0],
                                             start=True, stop=True)
                        else:
                            wk = wts[ch][li - 1]
                            T0, T1, T2 = X
                            for ki, xt in enumerate((T0, T1, T2)):
                                lhs = xt[:, pc * PC:(pc + 1) * PC]
                                nc.tensor.matmul(ps[:, o:o + 512],
                                                 lhsT=lhs, rhs=wk[ki][:, 0:512],
                                                 start=(ki == 0), stop=(ki == 2))
                                nc.tensor.matmul(ps[:, o + 512:o + NW],
                                                 lhsT=lhs, rhs=wk[ki][:, 512:NW],
                                                 start=(ki == 0), stop=(ki == 2))
                    ps3 = ps[:].rearrange("p (s w) -> p s w", w=1024)
                    g0 = ps3[:, :, 0:H]
                    k1 = kpool.tile([128, 2 * H], f32, tag=f"k1{ch}")
                    nc.vector.tensor_scalar(k1[:], g0, MAGIC, MAGIC,
                                            Alu.add, Alu.subtract)
                    nc.vector.scalar_tensor_tensor(
                        fw[:, t * 2 * H:(t + 1) * 2 * H], k1[:], -1.0, g0,
                        Alu.mult, Alu.add)
                    nc.scalar.activation(
                        sqw[:, t * 2 * gw:(t + 1) * 2 * gw],
                        ps3[:, :, H:H + gw], Act.Square, bias=0.0, scale=1.0)
                    if li > 0:
                        # per-tile v-adds over this tile's sq sections so
                        # they overlap the remaining matmuls
                        sq3t = sqw[:, t * 2 * gw:(t + 1) * 2 * gw].rearrange(
                            "p (s g) -> p s g", g=gw)
                        tsl = slice(t * 2 * H, (t + 1) * 2 * H)
                        nc.vector.tensor_tensor(v1w[:, tsl], sq3t[:, :, 0:H],
                                                sq3t[:, :, H:2 * H], Alu.add)
                        nc.gpsimd.tensor_tensor(v2w[:, tsl],
                                                sq3t[:, :, 2 * H:3 * H],
                                                sq3t[:, :, 3 * H:4 * H],
                                                Alu.add)
                    if t == NPC // 4 - 1:
                        # first-half Sins fire while the second half's
                        # matmuls stream
                        nc.scalar.activation(s[:, 0:HW2], fw[:, 0:HW2],
                                             Act.Sin, bias=0.0, scale=2 * PI)
                        nc.scalar.activation(sh[:, 0:HW2], fw[:, 0:HW2],
                                             Act.Sin, bias=0.0, scale=PI)
                nc.scalar.activation(s[:, HW2:W], fw[:, HW2:W], Act.Sin,
                                     bias=0.0, scale=2 * PI)
                nc.scalar.activation(sh[:, HW2:W], fw[:, HW2:W], Act.Sin,
                                     bias=0.0, scale=PI)
                # wv aliases the fw buffer: its WAR on the Sins matches the
                # Scalar queue order anyway
                wv = spool.tile([128, W], f16, tag=f"fw{ch}")
                if li == 0:
                    sq3 = sqw[:, 0:NPC * 2 * H].rearrange("p (n g) -> p n g",
                                                          g=2 * H)
                    nc.gpsimd.tensor_tensor(wv[:], sq3[:, :, 0:H],
                                            sq3[:, :, H:2 * H], Alu.add)
                else:
                    nc.gpsimd.tensor_tensor(wv[:], v1w[:], v2w[:], Alu.add)
                return s, sh, wv

            def expph(ch, wv):
                W = NPC * H
                E = spool.tile([128, W], f16, tag=f"E{ch}")
                nc.scalar.activation(E[:, 0:W // 2], wv[:, 0:W // 2], Act.Exp,
                                     bias=0.0, scale=-1.0)
                nc.scalar.activation(E[:, W // 2:W], wv[:, W // 2:W], Act.Exp,
                                     bias=0.0, scale=-1.0)
                return E

            def combine_transpose(ch, s, sh, E):
                """c2/xre/xim into xw in half-batches, each half's
                transposes following immediately; 3 psum evictions build
                the next layer's feature-major T tiles."""
                W = NPC * H
                c2 = spool.tile([128, W], f16, tag=f"v1{ch}")
                # xw aliases the sqw buffer (dead after the v-adds)
                xw = spool.tile([128, NPC * 363], f16, tag=f"sqw{ch}")
                x3 = xw[:].rearrange("p (n g) -> p n g", g=363)
                nc.vector.memset(x3[:, :, 2 * H:363], 1.0)
                T0 = xpool.tile([128, PB], f16, tag=f"T0{ch}", name=f"T0{ch}")
                T1 = xpool.tile([128, PB], f16, tag=f"T1{ch}", name=f"T1{ch}")
                T2 = xpool.tile([107, PB], f16, tag=f"T2{ch}", name=f"T2{ch}")
                tps = [pp.tile([c1 - c0, PB], f16, tag="wav",
                               name=f"tp{ki}{ch}")
                       for ki, (c0, c1) in enumerate(KC)]
                NH = NPC // 2
                for h in range(2):
                    n0, n1 = h * NH, (h + 1) * NH
                    hsl = slice(n0 * H, n1 * H)
                    nc.gpsimd.tensor_tensor(c2[:, hsl], sh[:, hsl], sh[:, hsl],
                                            Alu.mult)
                    nc.vector.scalar_tensor_tensor(
                        x3[:, n0:n1, 0:H], c2[:, hsl], 0.5, E[:, hsl],
                        Alu.subtract, Alu.mult)
                    nc.gpsimd.tensor_tensor(x3[:, n0:n1, H:2 * H], E[:, hsl],
                                            s[:, hsl], Alu.mult)
                    for ki, (c0, c1) in enumerate(KC):
                        if h == 1 and ki == 2:
                            nc.vector.tensor_copy(T0[:], tps[0][:])
                            nc.vector.tensor_copy(T1[:], tps[1][:])
                        for pc in range(n0, n1):
                            nc.tensor.transpose(
                                tps[ki][:, pc * PC:(pc + 1) * PC],
                                xw[:, pc * 363 + c0:pc * 363 + c1], idn[:])
                nc.vector.tensor_copy(T2[:], tps[2][:])
                return T0, T1, T2

            # ---- main loop -------------------------------------------
            def load_xa(nb):
                d = {}
                for ch in range(CPC):
                    t = apool.tile([3, PB], f16, tag=f"xa{ch}")
                    nc.sync.dma_start(out=t[:],
                                      in_=xa_d[ch, :, nb * PB:(nb + 1) * PB])
                    d[ch] = t
                return d

            X = {ch: None for ch in range(CPC)}
            xan = load_xa(0)
            for nb in range(NBATCH):
                for li in (0, 1, 2):
                    mid = {}
                    for ch in range(CPC):
                        mid[ch] = mm_early(ch, li, xan[ch], X[ch])
                    if li == 0 and nb + 1 < NBATCH:
                        xan_next = load_xa(nb + 1)
                    Ee = {}
                    for ch in range(CPC):
                        Ee[ch] = expph(ch, mid[ch][2])
                    for ch in range(CPC):
                        s, sh, _ = mid[ch]
                        X[ch] = combine_transpose(ch, s, sh, Ee[ch])
                # ---------- final linear (M=3, real part) ----------
                for ch in range(CPC):
                    T0, T1, T2 = X[ch]
                    psf = pp.tile([OUT, PB], f32, tag="wav")
                    for ni in range(PB // 512):
                        sl = slice(ni * 512, (ni + 1) * 512)
                        for ki, xt in enumerate((T0, T1, T2)):
                            nc.tensor.matmul(psf[:, sl], lhsT=wfts[ch][ki][:],
                                             rhs=xt[:, sl],
                                             start=(ki == 0), stop=(ki == 2))
                    ob = spool.tile([OUT, PB], f16, tag=f"ob{ch}")
                    nc.vector.tensor_copy(ob[:], psf[:])
                    nc.sync.dma_start(out=out_d[ch, :, nb * PB:(nb + 1) * PB],
                                      in_=ob[:])
                if nb + 1 < NBATCH:
                    xan = xan_next
    nc.finalize()
    return nc


def _get_graph():
    global _GRAPH
    if _GRAPH is None:
        _GRAPH = _build_graph()
    return _GRAPH


def _pack_inputs(inp, indices, model_idx, bias_idx, W0a, b0a, W0b, b0b,
                 W1a, b1a, W1b, b1b, W2a, b2a, W2b, b2b, Wf, bf):
    """Host-side gather + weight packing. Returns in_maps for 8 cores.

    Weight rows are input features in T-tile order [xre'(181); xim'(181);
    ones]; columns are the 4 group outputs [g0|g1|g2|g3].  alpha/beta are
    the affine factors mapping stored activations to true ones:
    x.re = alpha*xre', x.im = beta*xim'.
    """
    cplx = lambda a: a[..., 0] + 1j * a[..., 1]
    idn = np.eye(128, dtype=np.float16)

    def pack_hidden(Wa, Wb, ba, bb, alpha, beta):
        # columns [g0 | g1 | g2 | g3 | g4] with g4 = S0*g0 (so the
        # sum-of-squares pass covers (S*la.re)^2 too, no separate u0)
        re_rows = np.concatenate([
            R2 * alpha * Wa.real, SCALE * alpha * Wa.imag,
            SCALE * alpha * Wb.real, SCALE * alpha * Wb.imag,
            SCALE * alpha * Wa.real], axis=1)
        im_rows = np.concatenate([
            -R2 * beta * Wa.imag, SCALE * beta * Wa.real,
            -SCALE * beta * Wb.imag, SCALE * beta * Wb.real,
            -SCALE * beta * Wa.imag], axis=1)
        ones_row = np.concatenate([
            R2 * ba.real, SCALE * ba.imag + OMEGA / (2 * SCALE),
            SCALE * bb.real, SCALE * bb.imag, SCALE * ba.real])[None, :]
        return np.concatenate([re_rows, im_rows, ones_row],
                              axis=0).astype(np.float16)

    a0, b0c = -2.0, 1.0                          # L0 -> L1 factors
    a1, b1c = -2.0 * np.exp(EBIAS), np.exp(EBIAS)  # L1 -> L2, L2 -> final

    in_maps = []
    for core in range(NCORES):
        m = {k: [] for k in ("xa", "w0", "w1", "w2", "wf")}
        for j in range(CPC):
            c = core * CPC + j
            mi, bi = int(model_idx[c]), int(bias_idx[c])
            x = inp[int(indices[c])]          # [N, NIN]
            m["xa"].append(np.concatenate(
                [x.T, np.ones((1, N), np.float32)], 0).astype(np.float16))
            w0blk = np.concatenate([
                np.concatenate([R2 * W0a[mi], SCALE * W0b[mi],
                                SCALE * W0a[mi]], axis=1),
                np.concatenate([R2 * b0a[bi], SCALE * b0b[bi],
                                SCALE * b0a[bi]])[None, :],
            ], axis=0)
            m["w0"].append(w0blk.astype(np.float16))
            m["w1"].append(pack_hidden(cplx(W1a[mi]), cplx(W1b[mi]),
                                       cplx(b1a[bi]), cplx(b1b[bi]), a0, b0c))
            m["w2"].append(pack_hidden(cplx(W2a[mi]), cplx(W2b[mi]),
                                       cplx(b2a[bi]), cplx(b2b[bi]), a1, b1c))
            Wfc, bfc = cplx(Wf[mi]), cplx(bf[bi])
            wfblk = np.concatenate([
                a1 * Wfc.real, -b1c * Wfc.imag, bfc.real[None, :]],
                axis=0).astype(np.float16)
            m["wf"].append(wfblk)
        packed = {k: np.stack(v) for k, v in m.items()}
        packed["idn"] = idn
        in_maps.append(packed)
    return in_maps


def kernel(**inputs):
    inp = np.asarray(inputs["inp"], np.float32)
    args = {k: np.asarray(v) for k, v in inputs.items()}
    in_maps = _pack_inputs(
        inp, args["indices"], args["model_idx"], args["bias_idx"],
        *[np.asarray(args[k], np.float32) for k in
          ("W0a", "b0a", "W0b", "b0b", "W1a", "b1a", "W1b", "b1b",
           "W2a", "b2a", "W2b", "b2b", "Wf", "bf")])
    from concourse.bass_utils import run_bass_kernel_spmd
    nc = _get_graph()
    res = run_bass_kernel_spmd(nc, in_maps, core_ids=list(range(NCORES)))
    out = np.empty((1, C, N, OUT), np.float32)
    for core in range(NCORES):
        o = np.asarray(res.results[core]["out"])   # [CPC, OUT, N] fp16
        for j in range(CPC):
            out[0, core * CPC + j] = o[j].T.astype(np.float32)
    return out


if __name__ == "__main__":
    import jax
    import reference
    cpu = jax.devices("cpu")[0]
    with jax.default_device(cpu):
        ins = {k: np.asarray(v) for k, v in reference.setup_inputs().items()}
        exp = np.asarray(reference.reference(
            **{k: jax.device_put(v, cpu) for k, v in ins.items()}))
    got = kernel(**ins)
    rel = np.linalg.norm(got - exp) / np.linalg.norm(exp)
    print("Relative error:", rel)
